# revision 9
# baseline (speedup 1.0000x reference)
"""Trainium2 Bass kernel for a char-GRU:
  y = FC(GRU_last_hidden(Embed(x)))   with V=128, E=H=OUT=768, B=128, T=512.

Strategy (per core, data-parallel over batch, 8 cores x 16 rows):
  - table[v, :] = emb[v] @ W_ih.T + b_ih (+ b_hh for the r/z gate columns),
    computed on-device once.  Since V=128, the big input-side GEMM
    xe @ W_ih.T collapses into a row-gather from this [128, 2304] table.
  - The gather is done on the tensor engine: a one-hot [128v, 16b] stationary
    tile accumulates table rows directly into the gate PSUM banks.
  - 512 sequential GRU steps; per step the moving operand is W_hh^T
    (fp32r, 1 col/cycle), stationary is h^T (16 cols, cheap reload).
  - h_new = h + (1-z)*(n-h); (1-z) computed directly as sigmoid(-pre_z).
  - h_new [16, 768] is transposed back to h^T via 6 PE transposes.
"""

import os
import numpy as np
from contextlib import ExitStack

import concourse.bass as bass
import concourse.bacc as bacc
import concourse.tile as tile
from concourse import mybir
from concourse.bass_utils import run_bass_kernel_spmd

F32 = mybir.dt.float32
F32R = mybir.dt.float32r
I32 = mybir.dt.int32

V, E, H, OUT = 128, 768, 768, 768
G3 = 3 * H           # 2304
B_FULL, T_FULL = 128, 512
NCORES = 8
BS = B_FULL // NCORES  # 16
KT = H // 128          # 6 hidden k-tiles


def _bank_chunks(start, length):
    """Split [start, start+length) into pieces not crossing 512-elem banks."""
    cur, end = start, start + length
    while cur < end:
        w = min(512 - (cur % 512), end - cur)
        yield cur, w
        cur += w


def emit_kernel(ctx: ExitStack, tc: tile.TileContext, io: dict, T: int):
    nc = tc.nc
    add = mybir.AluOpType.add
    sub = mybir.AluOpType.subtract
    mult = mybir.AluOpType.mult
    iseq = mybir.AluOpType.is_equal
    Sig = mybir.ActivationFunctionType.Sigmoid
    Tanh = mybir.ActivationFunctionType.Tanh

    x_d, embT_d, wihT_d, whhT_d, bih_d, bhh_d, fcT_d, fcb_d, y_d = (
        io["x"], io["embT"], io["wihT"], io["whhT"], io["bih"], io["bhh"],
        io["fcT"], io["fcb"], io["y"],
    )

    consts = ctx.enter_context(tc.tile_pool(name="consts", bufs=1))

    # ---- persistent SBUF ----
    whhT_sb = consts.tile([128, KT, G3], F32R, name="whhT_sb")
    table_sb = consts.tile([128, G3], F32R, name="table_sb")
    onehot_sb = consts.tile([128, T * BS], F32R, name="onehot_sb")
    fcT_sb = consts.tile([128, KT, OUT], F32R, name="fcT_sb")
    fcb_sb = consts.tile([1, OUT], F32R, name="fcb_sb")
    bhh_sb = consts.tile([1, G3], F32R, name="bhh_sb")
    ones1b = consts.tile([1, BS], F32R, name="ones1b")
    ones1v = consts.tile([1, V], F32R, name="ones1v")
    ident16 = consts.tile([BS, BS], F32, name="ident16")
    iota_col = consts.tile([128, 1], F32, name="iota_col")

    for k in range(KT):
        nc.sync.dma_start(whhT_sb[:, k, :], whhT_d[k])
        nc.sync.dma_start(fcT_sb[:, k, :], fcT_d[k])
    nc.sync.dma_start(bhh_sb[:], bhh_d[:])
    nc.sync.dma_start(fcb_sb[:], fcb_d[:])

    # ---- tiny constants ----
    ones_f = consts.tile([1, V], F32, name="ones_f")
    nc.vector.memset(ones_f[:], 1.0)
    nc.scalar.copy(ones1v[:], ones_f[:])
    nc.scalar.copy(ones1b[:], ones_f[:, 0:BS])
    nc.gpsimd.iota(iota_col[:], pattern=[[0, 1]], base=0, channel_multiplier=1,
                   allow_small_or_imprecise_dtypes=True)

    ps_init = tc.alloc_tile_pool(name="ps_init", bufs=2, space="PSUM")

    # ---- phase A: one-hots (x scratch only) ----
    initA = tc.alloc_tile_pool(name="initA", bufs=1)
    xi_sb = initA.tile([1, T * BS], I32, name="xi_sb")
    xf_sb = initA.tile([1, T * BS], F32R, name="xf_sb")
    ones16 = initA.tile([BS, BS], F32, name="ones16")
    nc.sync.dma_start(xi_sb[:], x_d[:])
    nc.vector.memset(ones16[:], 1.0)
    # identity[p, f] = 1.0 where f == p
    nc.gpsimd.affine_select(ident16[:], ones16[:], pattern=[[1, BS]],
                            compare_op=iseq, fill=0.0, base=0,
                            channel_multiplier=-1)
    # x as float for the broadcast matmul
    nc.scalar.copy(xf_sb[:], xi_sb[:])
    # bcast x over partitions via K=1 matmul, compare against iota
    for c0 in range(0, T * BS, 512):
        w = min(512, T * BS - c0)
        psb = ps_init.tile([128, 512], F32, name="psb", tag="pst")
        nc.tensor.matmul(psb[:, 0:w], ones1v[:], xf_sb[:, c0:c0 + w],
                         start=True, stop=True)
        nc.vector.tensor_scalar(onehot_sb[:, c0:c0 + w], psb[:, 0:w],
                                iota_col[:], None, iseq)
    initA.release()

    # ---- phase B: table = embT.T @ wihT + biasrow ----
    initB = tc.alloc_tile_pool(name="initB", bufs=1)
    embT_sb = initB.tile([128, KT, V], F32R, name="embT_sb")
    wihT_sb = initB.tile([128, KT, G3], F32R, name="wihT_sb")
    biasrow_f = initB.tile([1, G3], F32, name="biasrow_f")
    biasrow = initB.tile([1, G3], F32R, name="biasrow")
    for k in range(KT):
        nc.sync.dma_start(embT_sb[:, k, :], embT_d[k])
        nc.sync.dma_start(wihT_sb[:, k, :], wihT_d[k])
    # biasrow = b_ih, plus b_hh on the r/z columns only
    nc.sync.dma_start(biasrow_f[:], bih_d[:])
    nc.vector.tensor_tensor(biasrow_f[:, 0:2 * H], biasrow_f[:, 0:2 * H],
                            bhh_sb[:, 0:2 * H].bitcast(F32), add)
    nc.scalar.copy(biasrow[:], biasrow_f[:])
    for c0 in range(0, G3, 512):
        w = min(512, G3 - c0)
        pst = ps_init.tile([V, 512], F32, name="pst", tag="pst")
        nc.tensor.matmul(pst[:, 0:w], ones1v[:], biasrow[:, c0:c0 + w],
                         start=True, stop=False)
        for k in range(KT):
            nc.tensor.matmul(pst[:, 0:w], embT_sb[:, k, :],
                             wihT_sb[:, k, c0:c0 + w],
                             start=False, stop=(k == KT - 1))
        nc.scalar.copy(table_sb[:, c0:c0 + w], pst[:, 0:w])
    initB.release()
    ps_init.release()

    # ---- step state ----
    state = ctx.enter_context(tc.tile_pool(name="state", bufs=1))
    h_pp = [state.tile([BS, H], F32, name=f"h_{i}") for i in range(2)]
    hT_pp = [state.tile([128, KT * BS], F32R, name=f"hT_{i}") for i in range(2)]

    tmp = ctx.enter_context(tc.tile_pool(name="tmp", bufs=2))
    ps = ctx.enter_context(tc.tile_pool(name="ps", bufs=1, space="PSUM"))

    HN0, XN0 = 0, H          # offsets inside ps_n: [hn(768) | xn(768)]
    for t in range(T):
        h_prev, h_new = h_pp[(t + 1) % 2], h_pp[t % 2]
        hT_prev, hT_new = hT_pp[(t + 1) % 2], hT_pp[t % 2]
        oh = onehot_sb[:, t * BS:(t + 1) * BS]

        ps_rz = ps.tile([BS, 2 * H], F32, name="ps_rz", tag="rz")
        ps_n = ps.tile([BS, 2 * H], F32, name="ps_n", tag="n")
        ps_hT = ps.tile([128, KT * BS], F32, name="ps_hT", tag="ht", bufs=2)

        # xn columns: pure gather (no h, no r)
        for c0, w in _bank_chunks(XN0, H):
            g0 = 2 * H + (c0 - XN0)
            nc.tensor.matmul(ps_n[:, c0:c0 + w], oh,
                             table_sb[:, g0:g0 + w],
                             start=True, stop=True)
        # r/z columns: gather + sum_k h^T_k @ W_hh^T_k
        for c0, w in _bank_chunks(0, 2 * H):
            nc.tensor.matmul(ps_rz[:, c0:c0 + w], oh,
                             table_sb[:, c0:c0 + w],
                             start=True, stop=(t == 0))
            if t > 0:
                for k in range(KT):
                    nc.tensor.matmul(
                        ps_rz[:, c0:c0 + w],
                        hT_prev[:, k * BS:(k + 1) * BS],
                        whhT_sb[:, k, c0:c0 + w],
                        start=False, stop=(k == KT - 1))
        # hn columns: b_hn row + sum_k h^T_k @ W_hn^T_k
        for c0, w in _bank_chunks(HN0, H):
            g0 = 2 * H + (c0 - HN0)
            nc.tensor.matmul(ps_n[:, c0:c0 + w], ones1b[:],
                             bhh_sb[:, g0:g0 + w],
                             start=True, stop=(t == 0))
            if t > 0:
                for k in range(KT):
                    nc.tensor.matmul(
                        ps_n[:, c0:c0 + w],
                        hT_prev[:, k * BS:(k + 1) * BS],
                        whhT_sb[:, k, g0:g0 + w],
                        start=False, stop=(k == KT - 1))

        # gates
        r_t = tmp.tile([BS, H], F32, name="r_t", tag="r")
        u_t = tmp.tile([BS, H], F32, name="u_t", tag="u")
        a_t = tmp.tile([BS, H], F32, name="a_t", tag="a")
        b_t = tmp.tile([BS, H], F32, name="b_t", tag="b")
        n_t = tmp.tile([BS, H], F32, name="n_t", tag="n")
        nc.scalar.activation(r_t[:], ps_rz[:, 0:H], Sig)
        nc.scalar.activation(u_t[:], ps_rz[:, H:2 * H], Sig, scale=-1.0)  # 1-z
        nc.vector.tensor_tensor(a_t[:], r_t[:], ps_n[:, HN0:HN0 + H], mult)
        nc.vector.tensor_tensor(b_t[:], a_t[:], ps_n[:, XN0:XN0 + H], add)
        nc.scalar.activation(n_t[:], b_t[:], Tanh)
        if t == 0:
            # h = 0  ->  h_new = (1-z) * n
            nc.vector.tensor_tensor(h_new[:], u_t[:], n_t[:], mult)
        else:
            c_t = tmp.tile([BS, H], F32, name="c_t", tag="c")
            d_t = tmp.tile([BS, H], F32, name="d_t", tag="d")
            nc.vector.tensor_tensor(c_t[:], n_t[:], h_prev[:], sub)
            nc.vector.tensor_tensor(d_t[:], u_t[:], c_t[:], mult)
            nc.vector.tensor_tensor(h_new[:], h_prev[:], d_t[:], add)

        # h^T via 6 PE transposes
        for k in range(KT):
            nc.tensor.transpose(ps_hT[:, k * BS:(k + 1) * BS],
                                h_new[:, k * 128:(k + 1) * 128], ident16[:])
        nc.scalar.copy(hT_new[:], ps_hT[:])

    # ---- FC head: y = h_T @ fc_W^T + fc_b ----
    hT_last = hT_pp[(T - 1) % 2]
    y_sb = consts.tile([BS, OUT], F32, name="y_sb")
    for c0 in range(0, OUT, 512):
        w = min(512, OUT - c0)
        ps_fc = ps.tile([BS, 512], F32, name="ps_fc", tag="rz")
        nc.tensor.matmul(ps_fc[:, 0:w], ones1b[:], fcb_sb[:, c0:c0 + w],
                         start=True, stop=False)
        for k in range(KT):
            nc.tensor.matmul(ps_fc[:, 0:w], hT_last[:, k * BS:(k + 1) * BS],
                             fcT_sb[:, k, c0:c0 + w],
                             start=False, stop=(k == KT - 1))
        nc.scalar.copy(y_sb[:, c0:c0 + w], ps_fc[:, 0:w])
    nc.sync.dma_start(y_d[:], y_sb[:])


def build(T: int = T_FULL, num_devices: int = NCORES):
    nc = bacc.Bacc("TRN2", target_bir_lowering=False, debug=False,
                   enable_asserts=False, num_devices=num_devices)
    io = {
        "x": nc.dram_tensor("x", [1, T * BS], I32, kind="ExternalInput").ap(),
        "embT": nc.dram_tensor("embT", [KT, 128, V], F32R,
                               kind="ExternalInput").ap(),
        "wihT": nc.dram_tensor("wihT", [KT, 128, G3], F32R,
                               kind="ExternalInput").ap(),
        "whhT": nc.dram_tensor("whhT", [KT, 128, G3], F32R,
                               kind="ExternalInput").ap(),
        "bih": nc.dram_tensor("bih", [1, G3], F32, kind="ExternalInput").ap(),
        "bhh": nc.dram_tensor("bhh", [1, G3], F32R, kind="ExternalInput").ap(),
        "fcT": nc.dram_tensor("fcT", [KT, 128, OUT], F32R,
                              kind="ExternalInput").ap(),
        "fcb": nc.dram_tensor("fcb", [1, OUT], F32R, kind="ExternalInput").ap(),
        "y": nc.dram_tensor("y", [BS, OUT], F32, kind="ExternalOutput").ap(),
    }
    with tile.TileContext(nc) as tc, ExitStack() as ctx:
        emit_kernel(ctx, tc, io, T)
    nc.compile()
    return nc


def make_in_maps(x, emb, W_ih, W_hh, b_ih, b_hh, fc_W, fc_b,
                 T: int = T_FULL, ncores: int = NCORES):
    x = np.asarray(x).astype(np.int32)[:, :T]
    emb = np.ascontiguousarray(np.asarray(emb, np.float32))
    embT = np.ascontiguousarray(emb.T).reshape(KT, 128, V)
    wihT = np.ascontiguousarray(np.asarray(W_ih, np.float32).T).reshape(
        KT, 128, G3)
    whhT = np.ascontiguousarray(np.asarray(W_hh, np.float32).T).reshape(
        KT, 128, G3)
    fcT = np.ascontiguousarray(np.asarray(fc_W, np.float32).T).reshape(
        KT, 128, OUT)
    bih = np.asarray(b_ih, np.float32).reshape(1, G3)
    bhh = np.asarray(b_hh, np.float32).reshape(1, G3)
    fcb = np.asarray(fc_b, np.float32).reshape(1, OUT)
    shared = {"embT": embT, "wihT": wihT, "whhT": whhT, "bih": bih,
              "bhh": bhh, "fcT": fcT, "fcb": fcb}
    in_maps = []
    for c in range(ncores):
        xs = x[c * BS:(c + 1) * BS]                       # [BS, T]
        x_tmaj = np.ascontiguousarray(xs.T).reshape(1, T * BS)  # t-major
        in_maps.append({"x": x_tmaj, **shared})
    return in_maps


_CACHE = {}


def kernel(x, emb, W_ih, W_hh, b_ih, b_hh, fc_W, fc_b):
    if "nc" not in _CACHE:
        _CACHE["nc"] = build()
    nc = _CACHE["nc"]
    in_maps = make_in_maps(x, emb, W_ih, W_hh, b_ih, b_hh, fc_W, fc_b)
    res = run_bass_kernel_spmd(nc, in_maps, core_ids=list(range(NCORES)))
    y = np.concatenate([res.results[c]["y"] for c in range(NCORES)], axis=0)
    return y.astype(np.float32)
